# revision 1
# baseline (speedup 1.0000x reference)
"""Bass/Trainium2 kernel for nn_NaryTreeLSTM (binary TreeLSTM over a complete
depth-16 tree, H=D=256, heap/level node order).

Sharding: data-parallel over 8 independent subtrees. Core m owns the subtree
rooted at level-3 node m; within every level l the core's nodes are a
contiguous position block whose children stay in the core's block at level
l+1 — zero inter-core communication. The device computes levels 15..CUT per
core; the tiny top of the tree (2^CUT-1 = 255 nodes, ~0.6% of FLOPs) is
finished on host during the gather/unshard step (the cross-core combine has
to leave the device at level 3 anyway; levels 6..3 are latency-bound serial
remnants that cost more in device sync than they are worth).

Layouts (per level, nodes stored in bit-reversed position order so the
even/odd children of a contiguous parent chunk are the first/second half of
the child level — no strided gathers):

- Big levels (n >= 256), "F-layout": feature-on-partition, nodes-on-free.
  W-tiles stationary, node columns moving (N<=512/matmul, float32r full
  rate). ACT applies sigmoid/tanh with the per-feature bias for free; DVE
  does the c/h elementwise work.

- Small levels (n <= 128), "N-layout": nodes-on-partition, weights moving.
  All 12 weight matrices stream through the PE as N=512 columns (float32r
  full rate regardless of node count); per-feature biases are added with a
  K=1 ones-row matmul. The h feedback for the next level is transposed back
  to feature-major via PE transposes.

Per node (children h_e,h_o / c_e,c_o; x = emb row):
  i = sig(Wi x + bi + Ui0 h_e + Ui1 h_o)      o, u analogous (u: tanh)
  f0 = sig(Wf x + bf + Uf0 h_e),  f1 = sig(Wf x + bf + Uf1 h_o)
  c = i*u + f0*c_e + f1*c_o ;  h = o * tanh(c)
"""

import os

import numpy as np

try:
    import concourse  # noqa: F401
except ImportError:  # pragma: no cover
    import sys

    sys.path.insert(0, "/opt/trn_rl_repo")

import concourse.tile as tile
from concourse import bacc, mybir
from concourse.bass_utils import run_bass_kernel_spmd

F32 = mybir.dt.float32
F32R = mybir.dt.float32r
AF = mybir.ActivationFunctionType

DEPTH = 16
H = 256
P = 128
NCORES = 8
LTOP = DEPTH - 1
CUT = 8  # device computes levels 15..CUT; host finishes 2^CUT-1 top nodes

N_L = {l: 1 << (l - 3) for l in range(CUT, LTOP + 1)}
NSLOT = sum(N_L.values())
OFF = {}
_o = 0
for _l in range(LTOP, CUT - 1, -1):
    OFF[_l] = _o
    _o += N_L[_l]
NOUT = N_L[CUT]

# F-layout weight tables: wta = [Wi, Wo, Wu] (needed by leaves, loaded first),
# wtb = [Ui0, Ui1, Uo0, Uo1, Uu0, Uu1, Wf, Uf0, Uf1]
W_I, W_O, W_U = 0, 1, 2
U_I0, U_I1, U_O0, U_O1, U_U0, U_U1, W_F, U_F0, U_F1 = range(9)
# N-layout packed weight columns (per ko, 12 blocks of 256 out-features):
# [Wi Wo Wu Wf Ui0 Uo0 Uu0 Uf0 Ui1 Uo1 Uu1 Uf1]
NL_A_X = (0, 512)  # -> psA = [i|o]
NL_B_X = (512, 1024)  # -> psB = [u|f0]
NL_C_X = (768, 1024)  # Wf -> psC = [f1]
NL_A_HE = (1024, 1536)
NL_B_HE = (1536, 2048)
NL_A_HO = (2048, 2560)
NL_BU_HO = (2560, 2816)  # Uu1 -> psB[:, 0:256]
NL_C_HO = (2816, 3072)
CHUNK = 512
NBLK = (NSLOT + 511) // 512


def _bitrev(nbits):
    n = 1 << nbits
    r = np.zeros(n, dtype=np.int64)
    for j in range(n):
        v = 0
        for b in range(nbits):
            if j & (1 << b):
                v |= 1 << (nbits - 1 - b)
        r[j] = v
    return r


def _build_program():
    nc = bacc.Bacc("TRN2", target_bir_lowering=False, debug=False, num_devices=NCORES)
    xtb = nc.dram_tensor("xtb", [NBLK, P, 2, 512], F32R, kind="ExternalInput").ap()
    wta = nc.dram_tensor("wta", [P, 3, 2, 2, P], F32R, kind="ExternalInput").ap()
    wtb = nc.dram_tensor("wtb", [P, 9, 2, 2, P], F32R, kind="ExternalInput").ap()
    wtn = nc.dram_tensor("wtn", [P, 2, 3072], F32R, kind="ExternalInput").ap()
    brow = nc.dram_tensor("brow", [1, 1280], F32R, kind="ExternalInput").ap()
    ones = nc.dram_tensor("ones", [1, P], F32R, kind="ExternalInput").ap()
    ident = nc.dram_tensor("ident", [P, P], F32, kind="ExternalInput").ap()
    bs = nc.dram_tensor("bs", [P, 4, 2], F32, kind="ExternalInput").ap()
    hc = nc.dram_tensor("hc", [NOUT, 2, H], F32, kind="ExternalOutput").ap()

    with tile.TileContext(nc) as tc:
        with (
            tc.tile_pool(name="const", bufs=1) as const,
            tc.tile_pool(name="xp", bufs=2) as xp,
            tc.tile_pool(name="fstate", bufs=1) as fstate,
            tc.tile_pool(name="ps6", bufs=6, space="PSUM") as ps6,
            tc.tile_pool(name="ps2", bufs=2, space="PSUM") as ps2,
        ):
            wta_sb = const.tile([P, 3, 2, 2, P], F32R)
            bs_sb = const.tile([P, 4, 2], F32)
            wtn_sb = const.tile([P, 2, 3072], F32R)
            ident_sb = const.tile([P, P], F32)
            brow_ones = {}

            def f_level(lvl, h_prev, c_prev, wtb_sb, fgp, fgp1, on_chunk=None):
                n = N_L[lvl]
                h_cur = fstate.tile([P, 2, n], F32R, tag=f"h{lvl % 2}", name="h")
                c_cur = fstate.tile([P, 2, n], F32, tag=f"c{lvl % 2}", name="c")
                for s in range(0, n, CHUNK):
                    ch = min(CHUNK, n - s)
                    e = s + ch
                    xt_t = xp.tile([P, 2, CHUNK], F32R, tag="x", name="x")
                    blk = (OFF[lvl] + s) // 512
                    w0 = (OFF[lvl] + s) % 512
                    nc.sync.dma_start(xt_t[:, :, :ch], xtb[blk][:, :, w0 : w0 + ch])
                    if on_chunk is not None:
                        on_chunk(s // CHUNK, c_cur, c_prev)
                    for mo in range(2):

                        def gate(srcs, g_idx, func, tag):
                            pt = ps6.tile([P, CHUNK], F32, tag="ps", name="ps")[:, :ch]
                            nmm = len(srcs) * 2
                            k = 0
                            for wsb, w_idx, rhs in srcs:
                                for ko in range(2):
                                    nc.tensor.matmul(
                                        pt,
                                        lhsT=wsb[:, w_idx, ko, mo],
                                        rhs=rhs(ko),
                                        start=(k == 0),
                                        stop=(k == nmm - 1),
                                    )
                                    k += 1
                            sb = fgp.tile([P, CHUNK], F32, tag=tag, name=tag)[:, :ch]
                            nc.scalar.activation(
                                sb, pt, func, bias=bs_sb[:, g_idx, mo : mo + 1]
                            )
                            return sb

                        def x_rhs(ko):
                            return xt_t[:, ko, :ch]

                        if lvl == LTOP:
                            i_sb = gate([(wta_sb, W_I, x_rhs)], 0, AF.Sigmoid, "gi")
                            u_sb = gate([(wta_sb, W_U, x_rhs)], 2, AF.Tanh, "gu")
                            o_sb = gate([(wta_sb, W_O, x_rhs)], 1, AF.Sigmoid, "go")
                            c_ap = c_cur[:, mo, s:e]
                            nc.vector.tensor_mul(out=c_ap, in0=i_sb, in1=u_sb)
                        else:
                            half = N_L[lvl + 1] // 2

                            def he(ko):
                                return h_prev[:, ko, s:e]

                            def ho(ko):
                                return h_prev[:, ko, half + s : half + e]

                            i_sb = gate(
                                [(wta_sb, W_I, x_rhs), (wtb_sb, U_I0, he), (wtb_sb, U_I1, ho)],
                                0, AF.Sigmoid, "gi",
                            )
                            u_sb = gate(
                                [(wta_sb, W_U, x_rhs), (wtb_sb, U_U0, he), (wtb_sb, U_U1, ho)],
                                2, AF.Tanh, "gu",
                            )
                            o_sb = gate(
                                [(wta_sb, W_O, x_rhs), (wtb_sb, U_O0, he), (wtb_sb, U_O1, ho)],
                                1, AF.Sigmoid, "go",
                            )
                            f0_sb = gate(
                                [(wtb_sb, W_F, x_rhs), (wtb_sb, U_F0, he)],
                                3, AF.Sigmoid, "f0",
                            )
                            f1_sb = gate(
                                [(wtb_sb, W_F, x_rhs), (wtb_sb, U_F1, ho)],
                                3, AF.Sigmoid, "f1",
                            )
                            ce = c_prev[:, mo, s:e]
                            co = c_prev[:, mo, half + s : half + e]
                            iu = fgp1.tile([P, CHUNK], F32, tag="iu", name="iu")[:, :ch]
                            nc.vector.tensor_mul(out=iu, in0=i_sb, in1=u_sb)
                            t0 = fgp1.tile([P, CHUNK], F32, tag="t0", name="t0")[:, :ch]
                            nc.vector.tensor_mul(out=t0, in0=f0_sb, in1=ce)
                            t1 = fgp1.tile([P, CHUNK], F32, tag="t1", name="t1")[:, :ch]
                            nc.vector.tensor_mul(out=t1, in0=f1_sb, in1=co)
                            c_ap = c_cur[:, mo, s:e]
                            nc.vector.tensor_add(out=c_ap, in0=iu, in1=t0)
                            nc.vector.tensor_add(out=c_ap, in0=c_ap, in1=t1)

                        th = fgp1.tile([P, CHUNK], F32, tag="th", name="th")[:, :ch]
                        nc.scalar.activation(th, c_ap, AF.Tanh)
                        nc.vector.tensor_mul(out=h_cur[:, mo, s:e], in0=o_sb, in1=th)
                return h_cur, c_cur

            def n_phase1(lvl, ngp):
                """x-projection + bias matmuls (no child dependency)."""
                n = N_L[lvl]
                xt_t = xp.tile([P, 2, CHUNK], F32R, tag="x", name="x")
                blk = OFF[lvl] // 512
                w0 = OFF[lvl] % 512
                nc.sync.dma_start(xt_t[:, :, :n], xtb[blk][:, :, w0 : w0 + n])
                psA = ps6.tile([P, CHUNK], F32, tag="ps", name="psA")
                psB = ps6.tile([P, CHUNK], F32, tag="ps", name="psB")
                psC = ps6.tile([P, CHUNK], F32, tag="ps", name="psC")
                for ko in range(2):
                    x_l = xt_t[:, ko, :n]
                    for pt, cols in (
                        (psA[:n, 0:512], NL_A_X),
                        (psB[:n, 0:512], NL_B_X),
                        (psC[:n, 0:256], NL_C_X),
                    ):
                        nc.tensor.matmul(
                            pt, lhsT=x_l, rhs=wtn_sb[:, ko, cols[0] : cols[1]],
                            start=(ko == 0), stop=False,
                        )
                nc.tensor.matmul(
                    psA[:n, 0:512], lhsT=brow_ones['ones'][:1, :n], rhs=brow_ones['brow'][:1, 0:512],
                    start=False, stop=False,
                )
                nc.tensor.matmul(
                    psB[:n, 0:512], lhsT=brow_ones['ones'][:1, :n], rhs=brow_ones['brow'][:1, 512:1024],
                    start=False, stop=False,
                )
                nc.tensor.matmul(
                    psC[:n, 0:256], lhsT=brow_ones['ones'][:1, :n], rhs=brow_ones['brow'][:1, 1024:1280],
                    start=False, stop=False,
                )
                return psA, psB, psC

            def n_phase2(lvl, ps3, lhs_he, lhs_ho, ce_nd, co_nd, ngp, nstate):
                """child matmuls + activations + elementwise + h transpose."""
                n = N_L[lvl]
                psA, psB, psC = ps3
                for ko in range(2):
                    he_l = lhs_he(ko)
                    ho_l = lhs_ho(ko)
                    last = ko == 1
                    for pt, cols, lh, st in (
                        (psA[:n, 0:512], NL_A_HE, he_l, False),
                        (psB[:n, 0:512], NL_B_HE, he_l, False),
                        (psA[:n, 0:512], NL_A_HO, ho_l, last),
                        (psB[:n, 0:256], NL_BU_HO, ho_l, last),
                        (psC[:n, 0:256], NL_C_HO, ho_l, last),
                    ):
                        nc.tensor.matmul(
                            pt, lhsT=lh, rhs=wtn_sb[:, ko, cols[0] : cols[1]],
                            start=False, stop=st,
                        )

                def act(pt_ap, func, tag):
                    sb = ngp.tile([P, H], F32, tag=tag, name=tag)[:n]
                    nc.scalar.activation(sb, pt_ap, func)
                    return sb

                # o is off the critical path to c -> emit last
                i_sb = act(psA[:n, 0:256], AF.Sigmoid, "ni")
                u_sb = act(psB[:n, 0:256], AF.Tanh, "nu")
                o_sb = act(psA[:n, 256:512], AF.Sigmoid, "no")
                f0_sb = act(psB[:n, 256:512], AF.Sigmoid, "nf0")
                f1_sb = act(psC[:n, 0:256], AF.Sigmoid, "nf1")

                h_small = nstate.tile([P, H], F32, tag=f"hs{lvl % 2}", name="hs")
                c_small = nstate.tile([P, H], F32, tag=f"cs{lvl % 2}", name="cs")
                iu = ngp.tile([P, H], F32, tag="niu", name="niu")[:n]
                nc.vector.tensor_mul(out=iu, in0=i_sb, in1=u_sb)
                t0 = ngp.tile([P, H], F32, tag="nt0", name="nt0")[:n]
                nc.vector.tensor_mul(out=t0, in0=f0_sb, in1=ce_nd)
                t1 = ngp.tile([P, H], F32, tag="nt1", name="nt1")[:n]
                nc.vector.tensor_mul(out=t1, in0=f1_sb, in1=co_nd)
                c_ap = c_small[:n]
                nc.vector.tensor_add(out=c_ap, in0=iu, in1=t0)
                nc.vector.tensor_add(out=c_ap, in0=c_ap, in1=t1)
                th = ngp.tile([P, H], F32, tag="nth", name="nth")[:n]
                nc.scalar.activation(th, c_ap, AF.Tanh)
                nc.vector.tensor_mul(out=h_small[:n], in0=o_sb, in1=th)

                hT = c_ev = c_od = None
                if lvl > CUT:
                    hT = nstate.tile([P, 2, P], F32R, tag=f"hT{lvl % 2}", name="hT")
                    for ko in range(2):
                        tr = ps2.tile([P, P], F32, tag="tr", name="tr")
                        nc.tensor.transpose(
                            tr[:, :n],
                            h_small[:n, ko * P : (ko + 1) * P],
                            ident_sb[:n, :n],
                        )
                        nc.vector.tensor_copy(hT[:, ko, :n], tr[:, :n])
                    # split c into base-0 even/odd halves for the parent
                    # (DVE tensor_tensor needs equal input base partitions)
                    half = n // 2
                    c_ev = nstate.tile([P, H], F32, tag=f"cev{lvl % 2}", name="cev")
                    c_od = nstate.tile([P, H], F32, tag=f"cod{lvl % 2}", name="cod")
                    nc.sync.dma_start(c_ev[:half], c_small[0:half])
                    nc.sync.dma_start(c_od[:half], c_small[half:n])
                return h_small, c_small, hT, c_ev, c_od

            # ---------------- tree walk ----------------
            with (
                tc.tile_pool(name="fwb", bufs=1) as fwb,
                tc.tile_pool(name="fgp", bufs=2) as fgp,
                tc.tile_pool(name="fgp1", bufs=2) as fgp1,
            ):
                wtb_sb = fwb.tile([P, 9, 2, 2, P], F32R)

                def on_chunk_leaf(ci, c_cur, c_prev):
                    # wta/bs are emitted right behind the first xt chunk so
                    # leaves start ASAP. The big U-table DMA is gated (WAW via
                    # a 1-elem copy that reads leaf c) so its HBM traffic
                    # cannot starve the leaf xt stream it would race with.
                    if ci == 0:
                        nc.sync.dma_start(wta_sb[:], wta)
                        nc.sync.dma_start(bs_sb[:], bs)
                    elif ci == 2:
                        nc.vector.tensor_copy(
                            wtb_sb[0:1, 0, 0, 0, 0:1], c_cur[0:1, 0, 0:1]
                        )
                        nc.sync.dma_start(wtb_sb[:], wtb)

                def on_chunk_l13(ci, c_cur, c_prev):
                    if ci == 0:
                        nc.vector.tensor_copy(
                            wtn_sb[0:1, 0, 0:1], c_prev[0:1, 0, 0:1]
                        )
                        nc.sync.dma_start(wtn_sb[:], wtn)
                        nc.sync.dma_start(ident_sb[:], ident)

                h_prev = c_prev = None
                lvl = LTOP
                while N_L[lvl] >= 256:
                    cb = on_chunk_leaf if lvl == LTOP else (
                        on_chunk_l13 if lvl == 13 else None
                    )
                    h_prev, c_prev = f_level(
                        lvl, h_prev, c_prev, wtb_sb, fgp, fgp1, on_chunk=cb
                    )
                    lvl -= 1

            with (
                tc.tile_pool(name="ngp", bufs=2) as ngp,
                tc.tile_pool(name="nstate", bufs=1) as nstate,
            ):
                brow_ones["brow"] = nstate.tile([1, 1280], F32R, name="browsb")
                nc.sync.dma_start(brow_ones["brow"][:], brow)
                brow_ones["ones"] = nstate.tile([1, P], F32R, name="onessb")
                nc.sync.dma_start(brow_ones["ones"][:], ones)
                n = N_L[lvl]  # 128
                ce_nd = nstate.tile([P, H], F32, tag="cnd0", name="cnd0")
                co_nd = nstate.tile([P, H], F32, tag="cnd1", name="cnd1")

                h_f = h_prev

                def he_b(ko):
                    return h_f[:, ko, 0:n]

                def ho_b(ko):
                    return h_f[:, ko, n : 2 * n]

                # phase-1 (child-independent) matmuls of the first two tail
                # levels go into the PE queue BEFORE the boundary transposes,
                # which must wait for the end of level 11.
                lvls = list(range(lvl, CUT - 1, -1))
                ps3 = {}
                ps3[lvls[0]] = n_phase1(lvls[0], ngp)
                if len(lvls) > 1:
                    ps3[lvls[1]] = n_phase1(lvls[1], ngp)

                # boundary: children of the first N-layout level are F-layout;
                # transpose child c (feature-major) to node-major.
                for ko in range(2):
                    for half, dst in ((0, ce_nd), (1, co_nd)):
                        tr = ps2.tile([P, P], F32, tag="tr", name="tr")
                        nc.tensor.transpose(
                            tr[:, :],
                            c_prev[:, ko, half * n : (half + 1) * n],
                            ident_sb[:, :],
                        )
                        nc.vector.tensor_copy(dst[:, ko * P : (ko + 1) * P], tr[:, :])

                h_small = c_small = hT = c_ev = c_od = None
                for k, l in enumerate(lvls):
                    if l == lvls[0]:
                        args = (he_b, ho_b, ce_nd[:n], co_nd[:n])
                    else:
                        nn = N_L[l]
                        hT_p, cev_p, cod_p = hT, c_ev, c_od

                        def he_n(ko, hT_p=hT_p, nn=nn):
                            return hT_p[:, ko, 0:nn]

                        def ho_n(ko, hT_p=hT_p, nn=nn):
                            return hT_p[:, ko, nn : 2 * nn]

                        args = (he_n, ho_n, cev_p[:nn], cod_p[:nn])
                    h_small, c_small, hT, c_ev, c_od = n_phase2(
                        l, ps3[l], *args, ngp, nstate
                    )
                    if k + 2 < len(lvls):
                        ps3[lvls[k + 2]] = n_phase1(lvls[k + 2], ngp)

                nc.sync.dma_start(hc[:, 0], h_small[:NOUT])
                nc.sync.dma_start(hc[:, 1], c_small[:NOUT])
    nc.compile()
    return nc


_CACHE = {}


def _get_program():
    if "nc" not in _CACHE:
        _CACHE["nc"] = _build_program()
    return _CACHE["nc"]


def _core_index_table():
    if "idx" in _CACHE:
        return _CACHE["idx"]
    idx = np.zeros((NCORES, NSLOT), dtype=np.int64)
    for lvl in range(LTOP, CUT - 1, -1):
        n = N_L[lvl]
        rev = _bitrev(lvl - 3)
        start = (1 << lvl) - 1
        for m in range(NCORES):
            pos = m * n + rev
            idx[m, OFF[lvl] : OFF[lvl] + n] = start + pos
    _CACHE["idx"] = idx
    return idx


def _pack_w(mat):
    """[out,in] (256,256) -> [p, ko, mo, m] = W.T[ko*128+p, mo*128+m]."""
    return mat.reshape(2, P, 2, P).transpose(3, 2, 0, 1)


def _sigmoid(x):
    return 1.0 / (1.0 + np.exp(-x))


def _host_node_batch(x, ch_h, ch_c, prm):
    (Wi, bi, Ui, Wo, bo, Uo, Wu, bu, Uu, Wf, bf, Uf) = prm

    def gate(W, b, U):
        return x @ W.T + b + ch_h[:, 0] @ U[0].T + ch_h[:, 1] @ U[1].T

    i = _sigmoid(gate(Wi, bi, Ui))
    o = _sigmoid(gate(Wo, bo, Uo))
    u = np.tanh(gate(Wu, bu, Uu))
    xf = x @ Wf.T + bf
    f0 = _sigmoid(xf + ch_h[:, 0] @ Uf[0].T)
    f1 = _sigmoid(xf + ch_h[:, 1] @ Uf[1].T)
    c = i * u + f0 * ch_c[:, 0] + f1 * ch_c[:, 1]
    h = o * np.tanh(c)
    return h.astype(np.float32), c.astype(np.float32)


def kernel(emb, W_i, b_i, U_i, W_o, b_o, U_o, W_u, b_u, U_u, W_f, b_f, U_f):
    emb = np.asarray(emb, dtype=np.float32)
    f = lambda a: np.asarray(a, dtype=np.float32)
    W_i, b_i, U_i = f(W_i), f(b_i), f(U_i)
    W_o, b_o, U_o = f(W_o), f(b_o), f(U_o)
    W_u, b_u, U_u = f(W_u), f(b_u), f(U_u)
    W_f, b_f, U_f = f(W_f), f(b_f), f(U_f)

    nc = _get_program()
    idx = _core_index_table()

    wta = np.ascontiguousarray(np.stack([_pack_w(m) for m in (W_i, W_o, W_u)], axis=1))
    wtb = np.ascontiguousarray(
        np.stack(
            [
                _pack_w(m)
                for m in (
                    U_i[0], U_i[1], U_o[0], U_o[1], U_u[0], U_u[1],
                    W_f, U_f[0], U_f[1],
                )
            ],
            axis=1,
        )
    )
    nl_mats = (
        W_i, W_o, W_u, W_f, U_i[0], U_o[0], U_u[0], U_f[0],
        U_i[1], U_o[1], U_u[1], U_f[1],
    )
    wtn = np.stack(
        [m.T.reshape(2, P, H).transpose(1, 0, 2) for m in nl_mats], axis=2
    )  # [p, ko, g, d]
    wtn = np.ascontiguousarray(wtn.reshape(P, 2, 12 * H))
    brow = np.zeros((1, 1280), dtype=np.float32)
    brow[0, 0:256] = b_i
    brow[0, 256:512] = b_o
    brow[0, 512:768] = b_u
    brow[0, 768:1024] = b_f
    brow[0, 1024:1280] = b_f
    ones = np.ones((1, P), dtype=np.float32)
    ident = np.eye(P, dtype=np.float32)
    bs = np.ascontiguousarray(
        np.stack([b.reshape(2, P).T for b in (b_i, b_o, b_u, b_f)], axis=1)
    )

    in_maps = []
    npad = NBLK * 512
    for m in range(NCORES):
        xm = emb[idx[m]]  # [NSLOT, 256]
        arr = np.zeros((256, npad), dtype=np.float32)
        arr[:, :NSLOT] = xm.T
        xtc = np.ascontiguousarray(
            arr.reshape(2, P, NBLK, 512).transpose(2, 1, 0, 3)
        )  # [blk, p, ko, s]
        in_maps.append(
            {
                "xtb": xtc, "wta": wta, "wtb": wtb, "wtn": wtn,
                "brow": brow, "ones": ones, "ident": ident, "bs": bs,
            }
        )

    kw = {}
    if os.environ.get("KERNEL_TRACE_DIR"):
        kw = {"trace": True, "tmpdir": os.environ["KERNEL_TRACE_DIR"]}
    res = run_bass_kernel_spmd(nc, in_maps, core_ids=list(range(NCORES)), **kw)
    _CACHE["last_res"] = res

    rev = _bitrev(CUT - 3)
    n_cut = 1 << CUT
    h = np.zeros((n_cut, H), dtype=np.float32)
    c = np.zeros((n_cut, H), dtype=np.float32)
    for m in range(NCORES):
        out = res.results[m]["hc"]  # [NOUT, 2, 256]
        pos = m * NOUT + rev
        h[pos] = out[:, 0]
        c[pos] = out[:, 1]

    prm = (W_i, b_i, U_i, W_o, b_o, U_o, W_u, b_u, U_u, W_f, b_f, U_f)
    for lvl in range(CUT - 1, -1, -1):
        n = 1 << lvl
        start = n - 1
        ch_h = h.reshape(n, 2, H)
        ch_c = c.reshape(n, 2, H)
        h, c = _host_node_batch(emb[start : start + n], ch_h, ch_c, prm)

    return h[0], c[0]



# revision 20
# speedup vs baseline: 1.0996x; 1.0996x over previous
"""Bass/Trainium2 kernel for nn_NaryTreeLSTM (binary TreeLSTM over a complete
depth-16 tree, H=D=256, heap/level node order).

Sharding: data-parallel over 8 independent subtrees. Core m owns the subtree
rooted at level-3 node m; within every level l the core's nodes are a
contiguous position block whose children stay in the core's block at level
l+1 — zero inter-core communication. The device computes levels 15..CUT per
core; the tiny top of the tree (2^CUT-1 = 255 nodes, ~0.6% of FLOPs) is
finished on host during the gather/unshard step (the cross-core combine has
to leave the device at level 3 anyway; levels 6..3 are latency-bound serial
remnants that cost more in device sync than they are worth).

Layouts (per level, nodes stored in bit-reversed position order so the
even/odd children of a contiguous parent chunk are the first/second half of
the child level — no strided gathers):

- Big levels (n >= 256), "F-layout": feature-on-partition, nodes-on-free.
  W-tiles stationary, node columns moving (N<=512/matmul, float32r full
  rate). ACT applies sigmoid/tanh with the per-feature bias for free; DVE
  does the c/h elementwise work.

- Small levels (n <= 128), "N-layout": nodes-on-partition, weights moving.
  All 12 weight matrices stream through the PE as N=512 columns (float32r
  full rate regardless of node count); per-feature biases are added with a
  K=1 ones-row matmul. The h feedback for the next level is transposed back
  to feature-major via PE transposes.

Per node (children h_e,h_o / c_e,c_o; x = emb row):
  i = sig(Wi x + bi + Ui0 h_e + Ui1 h_o)      o, u analogous (u: tanh)
  f0 = sig(Wf x + bf + Uf0 h_e),  f1 = sig(Wf x + bf + Uf1 h_o)
  c = i*u + f0*c_e + f1*c_o ;  h = o * tanh(c)
"""

import os

import ml_dtypes
import numpy as np

NP_BF16 = ml_dtypes.bfloat16

try:
    import concourse  # noqa: F401
except ImportError:  # pragma: no cover
    import sys

    sys.path.insert(0, "/opt/trn_rl_repo")

import concourse.tile as tile
from concourse import bacc, mybir
from concourse.bass_utils import run_bass_kernel_spmd

F32 = mybir.dt.float32
F32R = mybir.dt.float32r
BF16 = mybir.dt.bfloat16
AF = mybir.ActivationFunctionType

DEPTH = 16
H = 256
P = 128
NCORES = 8
LTOP = DEPTH - 1
CUT = 8  # device computes levels 15..CUT; host finishes 2^CUT-1 top nodes

N_L = {l: 1 << (l - 3) for l in range(CUT, LTOP + 1)}
NSLOT = sum(N_L.values())
OFF = {}
_o = 0
for _l in range(LTOP, CUT - 1, -1):
    OFF[_l] = _o
    _o += N_L[_l]
NOUT = N_L[CUT]

# F-layout weight tables: wta = [Wi, Wo, Wu] (needed by leaves, loaded first),
# wtb = [Ui0, Ui1, Uo0, Uo1, Uu0, Uu1, Wf, Uf0, Uf1]
W_I, W_O, W_U = 0, 1, 2
U_I0, U_I1, U_O0, U_O1, U_U0, U_U1, W_F, U_F0, U_F1 = range(9)
# N-layout packed weight columns (per ko, 12 blocks of 256 out-features):
# [Wi Wo Wu Wf Ui0 Uo0 Uu0 Uf0 Ui1 Uo1 Uu1 Uf1]
NL_A_X = (0, 512)  # -> psA = [i|o]
NL_B_X = (512, 1024)  # -> psB = [u|f0]
NL_C_X = (768, 1024)  # Wf -> psC = [f1]
NL_A_HE = (1024, 1536)
NL_B_HE = (1536, 2048)
NL_A_HO = (2048, 2560)
NL_BU_HO = (2560, 2816)  # Uu1 -> psB[:, 0:256]
NL_C_HO = (2816, 3072)
CHUNK = 512
NBLK = (NSLOT + 511) // 512


def _bitrev(nbits):
    n = 1 << nbits
    r = np.zeros(n, dtype=np.int64)
    for j in range(n):
        v = 0
        for b in range(nbits):
            if j & (1 << b):
                v |= 1 << (nbits - 1 - b)
        r[j] = v
    return r


def _build_program():
    nc = bacc.Bacc("TRN2", target_bir_lowering=False, debug=False, num_devices=NCORES)
    xtb = nc.dram_tensor("xtb", [NBLK, P, 2, 512], BF16, kind="ExternalInput").ap()
    wta = nc.dram_tensor("wta", [P, 3, 2, 2, P], BF16, kind="ExternalInput").ap()
    wtb = nc.dram_tensor("wtb", [P, 9, 2, 2, P], BF16, kind="ExternalInput").ap()
    wtn = nc.dram_tensor("wtn", [P, 2, 3072], BF16, kind="ExternalInput").ap()
    brow = nc.dram_tensor("brow", [1, 1280], BF16, kind="ExternalInput").ap()
    ones = nc.dram_tensor("ones", [1, P], BF16, kind="ExternalInput").ap()
    ident = nc.dram_tensor("ident", [P, P], F32, kind="ExternalInput").ap()
    identb = nc.dram_tensor("identb", [P, P], BF16, kind="ExternalInput").ap()
    bs = nc.dram_tensor("bs", [P, 4, 2], F32, kind="ExternalInput").ap()
    hc = nc.dram_tensor("hc", [NOUT, 2, H], F32, kind="ExternalOutput").ap()

    with tile.TileContext(nc) as tc:
        with (
            tc.tile_pool(name="const", bufs=1) as const,
            tc.tile_pool(name="xp", bufs=2) as xp,
            tc.tile_pool(name="fstate", bufs=1) as fstate,
            tc.tile_pool(name="ps6", bufs=6, space="PSUM") as ps6,
            tc.tile_pool(name="ps2", bufs=1, space="PSUM") as ps2,
        ):
            wta_sb = const.tile([P, 3, 2, 2, P], BF16)
            bs_sb = const.tile([P, 4, 2], F32)
            wtn_sb = const.tile([P, 2, 3072], BF16)
            ident_sb = const.tile([P, P], F32)
            identb_sb = const.tile([P, P], BF16)
            brow_ones = {}

            def f_level(lvl, h_prev, c_prev, wtb_sb, fgp, fgp1, on_chunk=None):
                n = N_L[lvl]
                h_cur = fstate.tile([P, 2, n], BF16, tag=f"h{lvl % 2}", name="h")
                c_cur = fstate.tile([P, 2, n], F32, tag=f"c{lvl % 2}", name="c")
                for s in range(0, n, CHUNK):
                    ch = min(CHUNK, n - s)
                    e = s + ch
                    xt_t = xp.tile([P, 2, CHUNK], BF16, tag="x", name="x")
                    blk = (OFF[lvl] + s) // 512
                    w0 = (OFF[lvl] + s) % 512
                    nc.sync.dma_start(xt_t[:, :, :ch], xtb[blk][:, :, w0 : w0 + ch])
                    if on_chunk is not None:
                        on_chunk(s // CHUNK, c_cur, c_prev)
                    for mo in range(2):

                        def gate(srcs, g_idx, func, tag):
                            pt = ps6.tile([P, CHUNK], F32, tag="ps", name="ps")[:, :ch]
                            nmm = len(srcs) * 2
                            k = 0
                            for wsb, w_idx, rhs in srcs:
                                for ko in range(2):
                                    nc.tensor.matmul(
                                        pt,
                                        lhsT=wsb[:, w_idx, ko, mo],
                                        rhs=rhs(ko),
                                        start=(k == 0),
                                        stop=(k == nmm - 1),
                                    )
                                    k += 1
                            sb = fgp.tile([P, CHUNK], BF16, tag=tag, name=tag)[:, :ch]
                            nc.scalar.activation(
                                sb, pt, func, bias=bs_sb[:, g_idx, mo : mo + 1]
                            )
                            return sb

                        def x_rhs(ko):
                            return xt_t[:, ko, :ch]

                        if lvl == LTOP:
                            i_sb = gate([(wta_sb, W_I, x_rhs)], 0, AF.Sigmoid, "gi")
                            u_sb = gate([(wta_sb, W_U, x_rhs)], 2, AF.Tanh, "gu")
                            o_sb = gate([(wta_sb, W_O, x_rhs)], 1, AF.Sigmoid, "go")
                            c_ap = c_cur[:, mo, s:e]
                            nc.vector.tensor_mul(out=c_ap, in0=i_sb, in1=u_sb)
                        else:
                            half = N_L[lvl + 1] // 2

                            def he(ko):
                                return h_prev[:, ko, s:e]

                            def ho(ko):
                                return h_prev[:, ko, half + s : half + e]

                            i_sb = gate(
                                [(wta_sb, W_I, x_rhs), (wtb_sb, U_I0, he), (wtb_sb, U_I1, ho)],
                                0, AF.Sigmoid, "gi",
                            )
                            u_sb = gate(
                                [(wta_sb, W_U, x_rhs), (wtb_sb, U_U0, he), (wtb_sb, U_U1, ho)],
                                2, AF.Tanh, "gu",
                            )
                            o_sb = gate(
                                [(wta_sb, W_O, x_rhs), (wtb_sb, U_O0, he), (wtb_sb, U_O1, ho)],
                                1, AF.Sigmoid, "go",
                            )
                            f0_sb = gate(
                                [(wtb_sb, W_F, x_rhs), (wtb_sb, U_F0, he)],
                                3, AF.Sigmoid, "f0",
                            )
                            f1_sb = gate(
                                [(wtb_sb, W_F, x_rhs), (wtb_sb, U_F1, ho)],
                                3, AF.Sigmoid, "f1",
                            )
                            ce = c_prev[:, mo, s:e]
                            co = c_prev[:, mo, half + s : half + e]
                            iu = fgp1.tile([P, CHUNK], F32, tag="iu", name="iu")[:, :ch]
                            nc.vector.tensor_mul(out=iu, in0=i_sb, in1=u_sb)
                            t0 = fgp1.tile([P, CHUNK], F32, tag="t0", name="t0")[:, :ch]
                            nc.vector.tensor_mul(out=t0, in0=f0_sb, in1=ce)
                            t1 = fgp1.tile([P, CHUNK], F32, tag="t1", name="t1")[:, :ch]
                            nc.vector.tensor_mul(out=t1, in0=f1_sb, in1=co)
                            c_ap = c_cur[:, mo, s:e]
                            nc.vector.tensor_add(out=c_ap, in0=iu, in1=t0)
                            nc.vector.tensor_add(out=c_ap, in0=c_ap, in1=t1)

                        th = fgp1.tile([P, CHUNK], BF16, tag="th", name="th")[:, :ch]
                        nc.scalar.activation(th, c_ap, AF.Tanh)
                        nc.vector.tensor_mul(out=h_cur[:, mo, s:e], in0=o_sb, in1=th)
                return h_cur, c_cur

            def n_phase1(lvl, ngp):
                """x-projection + bias matmuls (no child dependency)."""
                n = N_L[lvl]
                xt_t = xp.tile([P, 2, CHUNK], BF16, tag="x", name="x")
                blk = OFF[lvl] // 512
                w0 = OFF[lvl] % 512
                nc.sync.dma_start(xt_t[:, :, :n], xtb[blk][:, :, w0 : w0 + n])
                psA = ps6.tile([P, CHUNK], F32, tag="ps", name="psA")
                psB = ps6.tile([P, CHUNK], F32, tag="ps", name="psB")
                psC = ps6.tile([P, CHUNK], F32, tag="ps", name="psC")
                for ko in range(2):
                    x_l = xt_t[:, ko, :n]
                    for pt, cols in (
                        (psA[:n, 0:512], NL_A_X),
                        (psB[:n, 0:512], NL_B_X),
                        (psC[:n, 0:256], NL_C_X),
                    ):
                        nc.tensor.matmul(
                            pt, lhsT=x_l, rhs=wtn_sb[:, ko, cols[0] : cols[1]],
                            start=(ko == 0), stop=False,
                        )
                nc.tensor.matmul(
                    psA[:n, 0:512], lhsT=brow_ones['ones'][:1, :n], rhs=brow_ones['brow'][:1, 0:512],
                    start=False, stop=False,
                )
                nc.tensor.matmul(
                    psB[:n, 0:512], lhsT=brow_ones['ones'][:1, :n], rhs=brow_ones['brow'][:1, 512:1024],
                    start=False, stop=False,
                )
                nc.tensor.matmul(
                    psC[:n, 0:256], lhsT=brow_ones['ones'][:1, :n], rhs=brow_ones['brow'][:1, 1024:1280],
                    start=False, stop=False,
                )
                return psA, psB, psC

            def n_phase2(lvl, ps3, lhs_he, lhs_ho, ce_nd, co_nd, ngp, nstate):
                """child matmuls + activations + elementwise + h transpose."""
                n = N_L[lvl]
                psA, psB, psC = ps3
                for ko in range(2):
                    he_l = lhs_he(ko)
                    ho_l = lhs_ho(ko)
                    last = ko == 1
                    for pt, cols, lh, st in (
                        (psA[:n, 0:512], NL_A_HE, he_l, False),
                        (psB[:n, 0:512], NL_B_HE, he_l, False),
                        (psA[:n, 0:512], NL_A_HO, ho_l, last),
                        (psB[:n, 0:256], NL_BU_HO, ho_l, last),
                        (psC[:n, 0:256], NL_C_HO, ho_l, last),
                    ):
                        nc.tensor.matmul(
                            pt, lhsT=lh, rhs=wtn_sb[:, ko, cols[0] : cols[1]],
                            start=False, stop=st,
                        )

                def act(pt_ap, func, tag):
                    sb = ngp.tile([P, H], BF16, tag=tag, name=tag)[:n]
                    nc.scalar.activation(sb, pt_ap, func)
                    return sb

                # o is off the critical path to c -> emit last
                i_sb = act(psA[:n, 0:256], AF.Sigmoid, "ni")
                u_sb = act(psB[:n, 0:256], AF.Tanh, "nu")
                o_sb = act(psA[:n, 256:512], AF.Sigmoid, "no")
                f0_sb = act(psB[:n, 256:512], AF.Sigmoid, "nf0")
                f1_sb = act(psC[:n, 0:256], AF.Sigmoid, "nf1")

                h_small = nstate.tile([P, H], BF16, tag=f"hs{lvl % 2}", name="hs")
                c_small = nstate.tile([P, H], F32, tag=f"cs{lvl % 2}", name="cs")
                iu = ngp.tile([P, H], F32, tag="niu", name="niu")[:n]
                nc.vector.tensor_mul(out=iu, in0=i_sb, in1=u_sb)
                t0 = ngp.tile([P, H], F32, tag="nt0", name="nt0")[:n]
                nc.vector.tensor_mul(out=t0, in0=f0_sb, in1=ce_nd)
                t1 = ngp.tile([P, H], F32, tag="nt1", name="nt1")[:n]
                nc.vector.tensor_mul(out=t1, in0=f1_sb, in1=co_nd)
                c_ap = c_small[:n]
                nc.vector.tensor_add(out=c_ap, in0=iu, in1=t0)
                nc.vector.tensor_add(out=c_ap, in0=c_ap, in1=t1)
                th = ngp.tile([P, H], BF16, tag="nth", name="nth")[:n]
                nc.scalar.activation(th, c_ap, AF.Tanh)
                nc.vector.tensor_mul(out=h_small[:n], in0=o_sb, in1=th)

                hT = c_ev = c_od = None
                if lvl > CUT:
                    hT = nstate.tile([P, 2, P], BF16, tag=f"hT{lvl % 2}", name="hT")
                    for ko in range(2):
                        tr = ps2.tile([P, P], BF16, tag="trb", name="trb")
                        nc.tensor.transpose(
                            tr[:, :n],
                            h_small[:n, ko * P : (ko + 1) * P],
                            identb_sb[:n, :n],
                        )
                        nc.vector.tensor_copy(hT[:, ko, :n], tr[:, :n])
                    # split c into base-0 even/odd halves for the parent
                    # (DVE tensor_tensor needs equal input base partitions)
                    half = n // 2
                    c_ev = nstate.tile([P, H], F32, tag=f"cev{lvl % 2}", name="cev")
                    c_od = nstate.tile([P, H], F32, tag=f"cod{lvl % 2}", name="cod")
                    nc.sync.dma_start(c_ev[:half], c_small[0:half])
                    nc.sync.dma_start(c_od[:half], c_small[half:n])
                return h_small, c_small, hT, c_ev, c_od

            # ---------------- tree walk ----------------
            with (
                tc.tile_pool(name="fwb", bufs=1) as fwb,
                tc.tile_pool(name="fgp", bufs=2) as fgp,
                tc.tile_pool(name="fgp1", bufs=2) as fgp1,
            ):
                wtb_sb = fwb.tile([P, 9, 2, 2, P], BF16)

                def on_chunk_leaf(ci, c_cur, c_prev):
                    # wta/bs are emitted right behind the first xt chunk so
                    # leaves start ASAP. The big U-table DMA is gated (WAW via
                    # a 1-elem copy that reads leaf c) so its HBM traffic
                    # cannot starve the leaf xt stream it would race with.
                    if ci == 0:
                        nc.sync.dma_start(wta_sb[:], wta)
                        nc.sync.dma_start(bs_sb[:], bs)
                    elif ci == 2:
                        nc.vector.tensor_copy(
                            wtb_sb[0:1, 0, 0, 0, 0:1], c_cur[0:1, 0, 0:1]
                        )
                        nc.sync.dma_start(wtb_sb[:], wtb)

                def on_chunk_l13(ci, c_cur, c_prev):
                    if ci == 0:
                        nc.vector.tensor_copy(
                            wtn_sb[0:1, 0, 0:1], c_prev[0:1, 0, 0:1]
                        )
                        nc.sync.dma_start(wtn_sb[:], wtn)
                        nc.sync.dma_start(ident_sb[:], ident)
                        nc.sync.dma_start(identb_sb[:], identb)

                h_prev = c_prev = None
                lvl = LTOP
                while N_L[lvl] >= 256:
                    cb = on_chunk_leaf if lvl == LTOP else (
                        on_chunk_l13 if lvl == 13 else None
                    )
                    h_prev, c_prev = f_level(
                        lvl, h_prev, c_prev, wtb_sb, fgp, fgp1, on_chunk=cb
                    )
                    lvl -= 1

            with (
                tc.tile_pool(name="ngp", bufs=2) as ngp,
                tc.tile_pool(name="nstate", bufs=1) as nstate,
            ):
                brow_ones["brow"] = nstate.tile([1, 1280], BF16, name="browsb")
                nc.sync.dma_start(brow_ones["brow"][:], brow)
                brow_ones["ones"] = nstate.tile([1, P], BF16, name="onessb")
                nc.sync.dma_start(brow_ones["ones"][:], ones)
                n = N_L[lvl]  # 128
                ce_nd = nstate.tile([P, H], F32, tag="cnd0", name="cnd0")
                co_nd = nstate.tile([P, H], F32, tag="cnd1", name="cnd1")

                h_f = h_prev

                def he_b(ko):
                    return h_f[:, ko, 0:n]

                def ho_b(ko):
                    return h_f[:, ko, n : 2 * n]

                # phase-1 (child-independent) matmuls of the first two tail
                # levels go into the PE queue BEFORE the boundary transposes,
                # which must wait for the end of level 11.
                lvls = list(range(lvl, CUT - 1, -1))
                ps3 = {}
                ps3[lvls[0]] = n_phase1(lvls[0], ngp)
                if len(lvls) > 1:
                    ps3[lvls[1]] = n_phase1(lvls[1], ngp)

                # boundary: children of the first N-layout level are F-layout;
                # transpose child c (feature-major) to node-major.
                for ko in range(2):
                    for half, dst in ((0, ce_nd), (1, co_nd)):
                        tr = ps2.tile([P, P], F32, tag="tr", name="tr")
                        nc.tensor.transpose(
                            tr[:, :],
                            c_prev[:, ko, half * n : (half + 1) * n],
                            ident_sb[:, :],
                        )
                        nc.vector.tensor_copy(dst[:, ko * P : (ko + 1) * P], tr[:, :])

                h_small = c_small = hT = c_ev = c_od = None
                for k, l in enumerate(lvls):
                    if l == lvls[0]:
                        args = (he_b, ho_b, ce_nd[:n], co_nd[:n])
                    else:
                        nn = N_L[l]
                        hT_p, cev_p, cod_p = hT, c_ev, c_od

                        def he_n(ko, hT_p=hT_p, nn=nn):
                            return hT_p[:, ko, 0:nn]

                        def ho_n(ko, hT_p=hT_p, nn=nn):
                            return hT_p[:, ko, nn : 2 * nn]

                        args = (he_n, ho_n, cev_p[:nn], cod_p[:nn])
                    h_small, c_small, hT, c_ev, c_od = n_phase2(
                        l, ps3[l], *args, ngp, nstate
                    )
                    if k + 2 < len(lvls):
                        ps3[lvls[k + 2]] = n_phase1(lvls[k + 2], ngp)

                h32 = nstate.tile([P, H], F32, tag="h32", name="h32")
                nc.vector.tensor_copy(h32[:NOUT], h_small[:NOUT])
                nc.sync.dma_start(hc[:, 0], h32[:NOUT])
                nc.sync.dma_start(hc[:, 1], c_small[:NOUT])
    nc.compile()
    return nc


_CACHE = {}


def _get_program():
    if "nc" not in _CACHE:
        _CACHE["nc"] = _build_program()
    return _CACHE["nc"]


def _core_index_table():
    if "idx" in _CACHE:
        return _CACHE["idx"]
    idx = np.zeros((NCORES, NSLOT), dtype=np.int64)
    for lvl in range(LTOP, CUT - 1, -1):
        n = N_L[lvl]
        rev = _bitrev(lvl - 3)
        start = (1 << lvl) - 1
        for m in range(NCORES):
            pos = m * n + rev
            idx[m, OFF[lvl] : OFF[lvl] + n] = start + pos
    _CACHE["idx"] = idx
    return idx


def _pack_w(mat):
    """[out,in] (256,256) -> [p, ko, mo, m] = W.T[ko*128+p, mo*128+m]."""
    return mat.reshape(2, P, 2, P).transpose(3, 2, 0, 1)


def _sigmoid(x):
    return 1.0 / (1.0 + np.exp(-x))


def _host_node_batch(x, ch_h, ch_c, prm):
    (Wi, bi, Ui, Wo, bo, Uo, Wu, bu, Uu, Wf, bf, Uf) = prm

    def gate(W, b, U):
        return x @ W.T + b + ch_h[:, 0] @ U[0].T + ch_h[:, 1] @ U[1].T

    i = _sigmoid(gate(Wi, bi, Ui))
    o = _sigmoid(gate(Wo, bo, Uo))
    u = np.tanh(gate(Wu, bu, Uu))
    xf = x @ Wf.T + bf
    f0 = _sigmoid(xf + ch_h[:, 0] @ Uf[0].T)
    f1 = _sigmoid(xf + ch_h[:, 1] @ Uf[1].T)
    c = i * u + f0 * ch_c[:, 0] + f1 * ch_c[:, 1]
    h = o * np.tanh(c)
    return h.astype(np.float32), c.astype(np.float32)


def kernel(emb, W_i, b_i, U_i, W_o, b_o, U_o, W_u, b_u, U_u, W_f, b_f, U_f):
    emb = np.asarray(emb, dtype=np.float32)
    f = lambda a: np.asarray(a, dtype=np.float32)
    W_i, b_i, U_i = f(W_i), f(b_i), f(U_i)
    W_o, b_o, U_o = f(W_o), f(b_o), f(U_o)
    W_u, b_u, U_u = f(W_u), f(b_u), f(U_u)
    W_f, b_f, U_f = f(W_f), f(b_f), f(U_f)

    nc = _get_program()
    idx = _core_index_table()

    wta = np.ascontiguousarray(
        np.stack([_pack_w(m) for m in (W_i, W_o, W_u)], axis=1)
    ).astype(NP_BF16)
    wtb = np.ascontiguousarray(
        np.stack(
            [
                _pack_w(m)
                for m in (
                    U_i[0], U_i[1], U_o[0], U_o[1], U_u[0], U_u[1],
                    W_f, U_f[0], U_f[1],
                )
            ],
            axis=1,
        )
    ).astype(NP_BF16)
    nl_mats = (
        W_i, W_o, W_u, W_f, U_i[0], U_o[0], U_u[0], U_f[0],
        U_i[1], U_o[1], U_u[1], U_f[1],
    )
    wtn = np.stack(
        [m.T.reshape(2, P, H).transpose(1, 0, 2) for m in nl_mats], axis=2
    )  # [p, ko, g, d]
    wtn = np.ascontiguousarray(wtn.reshape(P, 2, 12 * H)).astype(NP_BF16)
    brow = np.zeros((1, 1280), dtype=np.float32)
    brow[0, 0:256] = b_i
    brow[0, 256:512] = b_o
    brow[0, 512:768] = b_u
    brow[0, 768:1024] = b_f
    brow[0, 1024:1280] = b_f
    brow = brow.astype(NP_BF16)
    ones = np.ones((1, P), dtype=NP_BF16)
    ident = np.eye(P, dtype=np.float32)
    identb = np.eye(P, dtype=NP_BF16)
    bs = np.ascontiguousarray(
        np.stack([b.reshape(2, P).T for b in (b_i, b_o, b_u, b_f)], axis=1)
    )

    in_maps = []
    npad = NBLK * 512
    for m in range(NCORES):
        xm = emb[idx[m]]  # [NSLOT, 256]
        arr = np.zeros((256, npad), dtype=np.float32)
        arr[:, :NSLOT] = xm.T
        xtc = np.ascontiguousarray(
            arr.reshape(2, P, NBLK, 512).transpose(2, 1, 0, 3)
        ).astype(NP_BF16)  # [blk, p, ko, s]
        in_maps.append(
            {
                "xtb": xtc, "wta": wta, "wtb": wtb, "wtn": wtn,
                "brow": brow, "ones": ones, "ident": ident, "identb": identb,
                "bs": bs,
            }
        )

    kw = {}
    if os.environ.get("KERNEL_TRACE_DIR"):
        kw = {"trace": True, "tmpdir": os.environ["KERNEL_TRACE_DIR"]}
    res = run_bass_kernel_spmd(nc, in_maps, core_ids=list(range(NCORES)), **kw)
    _CACHE["last_res"] = res

    rev = _bitrev(CUT - 3)
    n_cut = 1 << CUT
    h = np.zeros((n_cut, H), dtype=np.float32)
    c = np.zeros((n_cut, H), dtype=np.float32)
    for m in range(NCORES):
        out = res.results[m]["hc"]  # [NOUT, 2, 256]
        pos = m * NOUT + rev
        h[pos] = out[:, 0]
        c[pos] = out[:, 1]

    prm = (W_i, b_i, U_i, W_o, b_o, U_o, W_u, b_u, U_u, W_f, b_f, U_f)
    for lvl in range(CUT - 1, -1, -1):
        n = 1 << lvl
        start = n - 1
        ch_h = h.reshape(n, 2, H)
        ch_c = c.reshape(n, 2, H)
        h, c = _host_node_batch(emb[start : start + n], ch_h, ch_c, prm)

    return h[0], c[0]



# revision 22
# speedup vs baseline: 1.3024x; 1.1845x over previous
"""Bass/Trainium2 kernel for nn_NaryTreeLSTM (binary TreeLSTM over a complete
depth-16 tree, H=D=256, heap/level node order).

Sharding: data-parallel over 8 independent subtrees. Core m owns the subtree
rooted at level-3 node m; within every level l the core's nodes are a
contiguous position block whose children stay in the core's block at level
l+1 — zero inter-core communication. The device computes levels 15..CUT per
core in F-layout; the top of the tree (2^CUT-1 nodes, ~3% of FLOPs) is
finished on host during the gather/unshard step (those levels are
latency-bound serial remnants that cost more in device sync than they are
worth).

Layout (per level, nodes stored in bit-reversed position order so the
even/odd children of a contiguous parent chunk are the first/second half of
the child level — no strided gathers): feature-on-partition, nodes-on-free.
W-tiles stationary in bf16 (full-rate PE, half-cost LDWEIGHTS), node columns
moving (N<=512/matmul). ACT applies sigmoid/tanh with the per-feature bias;
DVE does the c/h elementwise work. c stays fp32; h and gates are bf16.

Per node (children h_e,h_o / c_e,c_o; x = emb row):
  i = sig(Wi x + bi + Ui0 h_e + Ui1 h_o)      o, u analogous (u: tanh)
  f0 = sig(Wf x + bf + Uf0 h_e),  f1 = sig(Wf x + bf + Uf1 h_o)
  c = i*u + f0*c_e + f1*c_o ;  h = o * tanh(c)
"""

import os

import ml_dtypes
import numpy as np

NP_BF16 = ml_dtypes.bfloat16

try:
    import concourse  # noqa: F401
except ImportError:  # pragma: no cover
    import sys

    sys.path.insert(0, "/opt/trn_rl_repo")

import concourse.tile as tile
from concourse import bacc, mybir
from concourse.bass_utils import run_bass_kernel_spmd

F32 = mybir.dt.float32
BF16 = mybir.dt.bfloat16
AF = mybir.ActivationFunctionType

DEPTH = 16
H = 256
P = 128
NCORES = 8
LTOP = DEPTH - 1
CUT = 11  # device computes levels 15..CUT; host finishes 2^CUT-1 top nodes

N_L = {l: 1 << (l - 3) for l in range(CUT, LTOP + 1)}
NSLOT = sum(N_L.values())
OFF = {}
_o = 0
for _l in range(LTOP, CUT - 1, -1):
    OFF[_l] = _o
    _o += N_L[_l]
NOUT = N_L[CUT]

# F-layout weight tables: wta = [Wi, Wo, Wu] (needed by leaves, loaded first),
# wtb = [Ui0, Ui1, Uo0, Uo1, Uu0, Uu1, Wf, Uf0, Uf1]
W_I, W_O, W_U = 0, 1, 2
U_I0, U_I1, U_O0, U_O1, U_U0, U_U1, W_F, U_F0, U_F1 = range(9)
CHUNK = 512
NBLK = (NSLOT + 511) // 512


def _bitrev(nbits):
    n = 1 << nbits
    r = np.zeros(n, dtype=np.int64)
    for j in range(n):
        v = 0
        for b in range(nbits):
            if j & (1 << b):
                v |= 1 << (nbits - 1 - b)
        r[j] = v
    return r


def _build_program():
    nc = bacc.Bacc("TRN2", target_bir_lowering=False, debug=False, num_devices=NCORES)
    xtb = nc.dram_tensor("xtb", [NBLK, P, 2, 512], BF16, kind="ExternalInput").ap()
    wta = nc.dram_tensor("wta", [P, 3, 2, 2, P], BF16, kind="ExternalInput").ap()
    wtb = nc.dram_tensor("wtb", [P, 9, 2, 2, P], BF16, kind="ExternalInput").ap()
    bs = nc.dram_tensor("bs", [P, 4, 2], F32, kind="ExternalInput").ap()
    hch = nc.dram_tensor("hch", [P, 2, NOUT], BF16, kind="ExternalOutput").ap()
    hcc = nc.dram_tensor("hcc", [P, 2, NOUT], F32, kind="ExternalOutput").ap()

    with tile.TileContext(nc) as tc:
        with (
            tc.tile_pool(name="const", bufs=1) as const,
            tc.tile_pool(name="xp", bufs=2) as xp,
            tc.tile_pool(name="fstate", bufs=1) as fstate,
            tc.tile_pool(name="ps6", bufs=8, space="PSUM") as ps6,
        ):
            wta_sb = const.tile([P, 3, 2, 2, P], BF16)
            bs_sb = const.tile([P, 4, 2], F32)

            def f_level(lvl, h_prev, c_prev, wtb_sb, fgp, fgp1, on_chunk=None):
                n = N_L[lvl]
                h_cur = fstate.tile([P, 2, n], BF16, tag=f"h{lvl % 2}", name="h")
                c_cur = fstate.tile([P, 2, n], F32, tag=f"c{lvl % 2}", name="c")
                for s in range(0, n, CHUNK):
                    ch = min(CHUNK, n - s)
                    e = s + ch
                    xt_t = xp.tile([P, 2, CHUNK], BF16, tag="x", name="x")
                    blk = (OFF[lvl] + s) // 512
                    w0 = (OFF[lvl] + s) % 512
                    nc.sync.dma_start(xt_t[:, :, :ch], xtb[blk][:, :, w0 : w0 + ch])
                    if on_chunk is not None:
                        on_chunk(s // CHUNK, c_cur, c_prev)
                    for mo in range(2):

                        def gate(srcs, g_idx, func, tag):
                            pt = ps6.tile([P, CHUNK], F32, tag="ps", name="ps")[:, :ch]
                            nmm = len(srcs) * 2
                            k = 0
                            for wsb, w_idx, rhs in srcs:
                                for ko in range(2):
                                    nc.tensor.matmul(
                                        pt,
                                        lhsT=wsb[:, w_idx, ko, mo],
                                        rhs=rhs(ko),
                                        start=(k == 0),
                                        stop=(k == nmm - 1),
                                    )
                                    k += 1
                            sb = fgp.tile([P, CHUNK], BF16, tag=tag, name=tag)[:, :ch]
                            nc.scalar.activation(
                                sb, pt, func, bias=bs_sb[:, g_idx, mo : mo + 1]
                            )
                            return sb

                        def x_rhs(ko):
                            return xt_t[:, ko, :ch]

                        if lvl == LTOP:
                            i_sb = gate([(wta_sb, W_I, x_rhs)], 0, AF.Sigmoid, "gi")
                            u_sb = gate([(wta_sb, W_U, x_rhs)], 2, AF.Tanh, "gu")
                            o_sb = gate([(wta_sb, W_O, x_rhs)], 1, AF.Sigmoid, "go")
                            c_ap = c_cur[:, mo, s:e]
                            nc.vector.tensor_mul(out=c_ap, in0=i_sb, in1=u_sb)
                        else:
                            half = N_L[lvl + 1] // 2

                            def he(ko):
                                return h_prev[:, ko, s:e]

                            def ho(ko):
                                return h_prev[:, ko, half + s : half + e]

                            i_sb = gate(
                                [(wta_sb, W_I, x_rhs), (wtb_sb, U_I0, he), (wtb_sb, U_I1, ho)],
                                0, AF.Sigmoid, "gi",
                            )
                            u_sb = gate(
                                [(wta_sb, W_U, x_rhs), (wtb_sb, U_U0, he), (wtb_sb, U_U1, ho)],
                                2, AF.Tanh, "gu",
                            )
                            o_sb = gate(
                                [(wta_sb, W_O, x_rhs), (wtb_sb, U_O0, he), (wtb_sb, U_O1, ho)],
                                1, AF.Sigmoid, "go",
                            )
                            f0_sb = gate(
                                [(wtb_sb, W_F, x_rhs), (wtb_sb, U_F0, he)],
                                3, AF.Sigmoid, "f0",
                            )
                            f1_sb = gate(
                                [(wtb_sb, W_F, x_rhs), (wtb_sb, U_F1, ho)],
                                3, AF.Sigmoid, "f1",
                            )
                            ce = c_prev[:, mo, s:e]
                            co = c_prev[:, mo, half + s : half + e]
                            iu = fgp1.tile([P, CHUNK], F32, tag="iu", name="iu")[:, :ch]
                            nc.vector.tensor_mul(out=iu, in0=i_sb, in1=u_sb)
                            t0 = fgp1.tile([P, CHUNK], F32, tag="t0", name="t0")[:, :ch]
                            nc.vector.tensor_mul(out=t0, in0=f0_sb, in1=ce)
                            t1 = fgp1.tile([P, CHUNK], F32, tag="t1", name="t1")[:, :ch]
                            nc.vector.tensor_mul(out=t1, in0=f1_sb, in1=co)
                            c_ap = c_cur[:, mo, s:e]
                            nc.vector.tensor_add(out=c_ap, in0=iu, in1=t0)
                            nc.vector.tensor_add(out=c_ap, in0=c_ap, in1=t1)

                        th = fgp1.tile([P, CHUNK], BF16, tag="th", name="th")[:, :ch]
                        nc.scalar.activation(th, c_ap, AF.Tanh)
                        nc.vector.tensor_mul(out=h_cur[:, mo, s:e], in0=o_sb, in1=th)
                return h_cur, c_cur

            # ---------------- tree walk ----------------
            with (
                tc.tile_pool(name="fwb", bufs=1) as fwb,
                tc.tile_pool(name="fgp", bufs=2) as fgp,
                tc.tile_pool(name="fgp1", bufs=2) as fgp1,
            ):
                wtb_sb = fwb.tile([P, 9, 2, 2, P], BF16)

                def on_chunk_leaf(ci, c_cur, c_prev):
                    # wta/bs are emitted right behind the first xt chunk so
                    # leaves start ASAP. The big U-table DMA is gated (WAW via
                    # a 1-elem copy that reads leaf c) so its HBM traffic
                    # cannot starve the leaf xt stream it would race with.
                    if ci == 0:
                        nc.sync.dma_start(wta_sb[:], wta)
                        nc.sync.dma_start(bs_sb[:], bs)
                    elif ci == 2:
                        nc.vector.tensor_copy(
                            wtb_sb[0:1, 0, 0, 0, 0:1], c_cur[0:1, 0, 0:1]
                        )
                        nc.sync.dma_start(wtb_sb[:], wtb)

                h_prev = c_prev = None
                for lvl in range(LTOP, CUT - 1, -1):
                    cb = on_chunk_leaf if lvl == LTOP else None
                    h_prev, c_prev = f_level(
                        lvl, h_prev, c_prev, wtb_sb, fgp, fgp1, on_chunk=cb
                    )

                nc.sync.dma_start(hch, h_prev[:, :, :])
                nc.sync.dma_start(hcc, c_prev[:, :, :])
    nc.compile()
    return nc


_CACHE = {}


def _get_program():
    if "nc" not in _CACHE:
        _CACHE["nc"] = _build_program()
    return _CACHE["nc"]


def _core_index_table():
    if "idx" in _CACHE:
        return _CACHE["idx"]
    idx = np.zeros((NCORES, NSLOT), dtype=np.int64)
    for lvl in range(LTOP, CUT - 1, -1):
        n = N_L[lvl]
        rev = _bitrev(lvl - 3)
        start = (1 << lvl) - 1
        for m in range(NCORES):
            pos = m * n + rev
            idx[m, OFF[lvl] : OFF[lvl] + n] = start + pos
    _CACHE["idx"] = idx
    return idx


def _pack_w(mat):
    """[out,in] (256,256) -> [p, ko, mo, m] = W.T[ko*128+p, mo*128+m]."""
    return mat.reshape(2, P, 2, P).transpose(3, 2, 0, 1)


def _sigmoid(x):
    return 1.0 / (1.0 + np.exp(-x))


def _host_node_batch(x, ch_h, ch_c, prm):
    (Wi, bi, Ui, Wo, bo, Uo, Wu, bu, Uu, Wf, bf, Uf) = prm

    def gate(W, b, U):
        return x @ W.T + b + ch_h[:, 0] @ U[0].T + ch_h[:, 1] @ U[1].T

    i = _sigmoid(gate(Wi, bi, Ui))
    o = _sigmoid(gate(Wo, bo, Uo))
    u = np.tanh(gate(Wu, bu, Uu))
    xf = x @ Wf.T + bf
    f0 = _sigmoid(xf + ch_h[:, 0] @ Uf[0].T)
    f1 = _sigmoid(xf + ch_h[:, 1] @ Uf[1].T)
    c = i * u + f0 * ch_c[:, 0] + f1 * ch_c[:, 1]
    h = o * np.tanh(c)
    return h.astype(np.float32), c.astype(np.float32)


def kernel(emb, W_i, b_i, U_i, W_o, b_o, U_o, W_u, b_u, U_u, W_f, b_f, U_f):
    emb = np.asarray(emb, dtype=np.float32)
    f = lambda a: np.asarray(a, dtype=np.float32)
    W_i, b_i, U_i = f(W_i), f(b_i), f(U_i)
    W_o, b_o, U_o = f(W_o), f(b_o), f(U_o)
    W_u, b_u, U_u = f(W_u), f(b_u), f(U_u)
    W_f, b_f, U_f = f(W_f), f(b_f), f(U_f)

    nc = _get_program()
    idx = _core_index_table()

    wta = np.ascontiguousarray(
        np.stack([_pack_w(m) for m in (W_i, W_o, W_u)], axis=1)
    ).astype(NP_BF16)
    wtb = np.ascontiguousarray(
        np.stack(
            [
                _pack_w(m)
                for m in (
                    U_i[0], U_i[1], U_o[0], U_o[1], U_u[0], U_u[1],
                    W_f, U_f[0], U_f[1],
                )
            ],
            axis=1,
        )
    ).astype(NP_BF16)
    bs = np.ascontiguousarray(
        np.stack([b.reshape(2, P).T for b in (b_i, b_o, b_u, b_f)], axis=1)
    )

    in_maps = []
    npad = NBLK * 512
    for m in range(NCORES):
        xm = emb[idx[m]]  # [NSLOT, 256]
        arr = np.zeros((256, npad), dtype=np.float32)
        arr[:, :NSLOT] = xm.T
        xtc = np.ascontiguousarray(
            arr.reshape(2, P, NBLK, 512).transpose(2, 1, 0, 3)
        ).astype(NP_BF16)  # [blk, p, ko, s]
        in_maps.append({"xtb": xtc, "wta": wta, "wtb": wtb, "bs": bs})

    kw = {}
    if os.environ.get("KERNEL_TRACE_DIR"):
        kw = {"trace": True, "tmpdir": os.environ["KERNEL_TRACE_DIR"]}
    res = run_bass_kernel_spmd(nc, in_maps, core_ids=list(range(NCORES)), **kw)
    _CACHE["last_res"] = res

    rev = _bitrev(CUT - 3)
    n_cut = 1 << CUT
    h = np.zeros((n_cut, H), dtype=np.float32)
    c = np.zeros((n_cut, H), dtype=np.float32)
    for m in range(NCORES):
        # [P, 2, NOUT] feature-major -> [NOUT, 2*P] node-major
        hm = np.asarray(res.results[m]["hch"]).astype(np.float32)
        cm = np.asarray(res.results[m]["hcc"])
        pos = m * NOUT + rev
        h[pos] = hm.transpose(2, 1, 0).reshape(NOUT, H)
        c[pos] = cm.transpose(2, 1, 0).reshape(NOUT, H)

    prm = (W_i, b_i, U_i, W_o, b_o, U_o, W_u, b_u, U_u, W_f, b_f, U_f)
    for lvl in range(CUT - 1, -1, -1):
        n = 1 << lvl
        start = n - 1
        ch_h = h.reshape(n, 2, H)
        ch_c = c.reshape(n, 2, H)
        h, c = _host_node_batch(emb[start : start + n], ch_h, ch_c, prm)

    return h[0], c[0]
